# revision 11
# baseline (speedup 1.0000x reference)
"""DreamAttention (GQA + RoPE + causal) on 8 trn2 NeuronCores.

Sharding: TP=4 over heads (7 q-heads + 1 kv-head per rank) x DP=2 over batch.
Core c -> (batch b = c // 4, tp rank r = c % 4).

Per-core dataflow (all matmuls in fp32r = full-rate ~tf32 precision):
  - host supplies xT (hidden transposed, D on partitions) tiled per s-chunk
  - projections: QT/KT/VT in [d_part, s_free] layout (W tile stationary, xT moving)
  - RoPE via SBUF->SBUF DMA partition rotation + DVE mul/add with host cos/sin tables
  - V^T -> V via PE transpose (16 tiles)
  - attention in transposed form: S^T[k,q] tiles -> exp (ACT, 1/sqrt(d) folded into
    host-prescaled Wq) -> PV accumulates out^T[d,q]; causal mask added on diagonal
    k-tiles; softmax sums via ones-matmul over a DVE-accumulated P^T running sum;
    normalization fused into the PSUM->SBUF move, which overwrites the spent Q slice
  - o_proj: attnT stationary, Wo moving, accumulate over the 7 head-chunks
  - ReduceScatter(add) over each 4-core TP group per 512-wide D chunk
Host reassembles the 8 cores' RS shards into the full [2, 2048, 3584] output.
"""

import math

import numpy as np

import concourse.bass as bass
import concourse.mybir as mybir
import concourse.tile as tile
from concourse import bacc
from concourse.bass_utils import run_bass_kernel_spmd
from concourse.masks import make_identity

F32 = mybir.dt.float32
F32R = mybir.dt.float32r

B, S, D = 2, 2048, 3584
H, KVH, HD = 28, 4, 128
ROPE_THETA = 1000000.0
NH = 7          # q heads per core
DKT = D // 128  # 28 k-tiles over D
SC = 512        # s-chunk width in phase A
NSC = S // SC   # 4
QB = 512        # q block width in attention
NQB = S // QB   # 4
NKT = S // 128  # 16 k tiles over sequence
NDC = 7         # output D chunks of 512
SCALE = 1.0 / math.sqrt(HD)

_NC_CACHE = {}


def _build_nc(loop_n=1, no_cc=False, phases="ABC"):
    import contextlib

    key = ("nc", loop_n, no_cc, phases)
    if key in _NC_CACHE:
        return _NC_CACHE[key]

    nc = bacc.Bacc("TRN2", target_bir_lowering=False, debug=False, num_devices=8)

    xt_d = nc.dram_tensor("xt", [NSC, DKT, 128, SC], F32R, kind="ExternalInput").ap()
    wq_d = nc.dram_tensor("wq", [NH, 128, DKT, 128], F32R, kind="ExternalInput").ap()
    wk_d = nc.dram_tensor("wk", [128, DKT, 128], F32R, kind="ExternalInput").ap()
    wv_d = nc.dram_tensor("wv", [128, DKT, 128], F32R, kind="ExternalInput").ap()
    wo_d = nc.dram_tensor("wo", [NDC, NH, 128, 512], F32R, kind="ExternalInput").ap()
    cos_d = nc.dram_tensor("cos_t", [128, S], F32R, kind="ExternalInput").ap()
    sin_d = nc.dram_tensor("sin_t", [128, S], F32R, kind="ExternalInput").ap()
    mask_d = nc.dram_tensor("mask", [4, 128, QB], F32, kind="ExternalInput").ap()
    rs_out = nc.dram_tensor("rs_out", [NDC, 512, 512], F32, kind="ExternalOutput").ap()

    with tile.TileContext(nc) as tc:
        with (
            tc.tile_pool(name="persist", bufs=1) as persist,
            tc.tile_pool(name="ps_proj", bufs=2, space="PSUM") as ps_proj,
            tc.tile_pool(name="ps_s", bufs=3, space="PSUM") as ps_s,
            tc.tile_pool(name="ps_o", bufs=2, space="PSUM") as ps_o,
            tc.tile_pool(name="ps_sum", bufs=1, space="PSUM") as ps_sum,
            tc.tile_pool(name="dram", bufs=1, space="DRAM") as dram,
        ):
            # qt doubles as the attention-output buffer: att(h, qblock)
            # overwrites qt[:, h, qblock] once those q columns are consumed.
            qt = persist.tile([128, NH, S], F32R, name="qt")
            kt = persist.tile([128, S], F32R, name="kt")
            vn = persist.tile([128, NKT, 128], F32R, name="vn")
            mask_t = persist.tile([128, 4, QB], F32, name="mask_t")
            ident = persist.tile([128, 128], F32, name="ident")
            ones = persist.tile([128, 1], F32R, name="ones")
            ones_f = persist.tile([128, 1], F32, name="ones_f")

            nc.sync.dma_start(out=mask_t, in_=mask_d.rearrange("r p q -> p r q"))
            make_identity(nc, ident)
            nc.vector.memset(ones_f, 1.0)
            nc.vector.tensor_copy(ones, ones_f)

            # Optional on-device repetition of the whole body (benchmarking
            # only: amortizes host dispatch overhead across loop_n runs).
            for _rep in range(loop_n):
                _body_phases(nc, tc, locals(), no_cc=no_cc, phases=phases)

    nc.finalize()
    _NC_CACHE[key] = nc
    return nc


def _body_phases(nc, tc, env, no_cc=False, phases="ABC"):
    qt = env["qt"]
    kt = env["kt"]
    vn = env["vn"]
    mask_t = env["mask_t"]
    ident = env["ident"]
    ones = env["ones"]
    xt_d = env["xt_d"]
    wq_d = env["wq_d"]
    wk_d = env["wk_d"]
    wv_d = env["wv_d"]
    wo_d = env["wo_d"]
    cos_d = env["cos_d"]
    sin_d = env["sin_d"]
    rs_out = env["rs_out"]
    ps_proj = env["ps_proj"]
    ps_s = env["ps_s"]
    ps_o = env["ps_o"]
    ps_sum = env["ps_sum"]
    dram = env["dram"]

    if True:
        if True:
            # ---- Phase A: projections (QT/KT/VT in [d, s] layout) ----
            with (
                tc.tile_pool(name="xtp", bufs=30) as xtp,
                tc.tile_pool(name="wp", bufs=2) as wp,
                tc.tile_pool(name="pha", bufs=1) as pha,
                tc.tile_pool(name="ropep", bufs=3) as ropep,
            ):
                vt = pha.tile([128, S], F32, name="vt")
                cos_t = pha.tile([128, S], F32R, name="cos_t")
                sin_t = pha.tile([128, S], F32R, name="sin_t")
                nc.sync.dma_start(out=cos_t, in_=cos_d)
                nc.sync.dma_start(out=sin_t, in_=sin_d)

                for sc in range(NSC):
                    xts = []
                    for kti in range(DKT):
                        xtile = xtp.tile([128, SC], F32R, name="xt")
                        nc.sync.dma_start(out=xtile, in_=xt_d[sc, kti])
                        xts.append(xtile)
                    for ct in range(NH + 2):
                        wblk = wp.tile([128, DKT, 128], F32R, name="wq")
                        if ct < NH:
                            nc.sync.dma_start(out=wblk, in_=wq_d[ct])
                            dest = qt[:, ct, sc * SC : (sc + 1) * SC]
                        elif ct == NH:
                            nc.sync.dma_start(out=wblk, in_=wk_d)
                            dest = kt[:, sc * SC : (sc + 1) * SC]
                        else:
                            nc.sync.dma_start(out=wblk, in_=wv_d)
                            dest = vt[:, sc * SC : (sc + 1) * SC]
                        psum = ps_proj.tile([128, SC], F32, name="pp")
                        for kti in range(DKT):
                            nc.tensor.matmul(
                                psum,
                                wblk[:, kti, :],
                                xts[kti],
                                start=(kti == 0),
                                stop=(kti == DKT - 1),
                            )
                        nc.vector.tensor_copy(dest, psum)

                # RoPE on KT and each QT head (partition rotation by 64),
                # chunked by 512 to keep the tmp pool small.
                def rope(dst):
                    for c in range(NSC):
                        sl = slice(c * SC, (c + 1) * SC)
                        tmp = ropep.tile([128, SC], F32R, name="ropetmp")
                        nc.sync.dma_start(out=tmp[0:64, :], in_=dst[64:128, sl])
                        nc.sync.dma_start(out=tmp[64:128, :], in_=dst[0:64, sl])
                        nc.vector.tensor_mul(tmp, tmp, sin_t[:, sl])
                        nc.vector.tensor_mul(dst[:, sl], dst[:, sl], cos_t[:, sl])
                        nc.vector.tensor_add(dst[:, sl], dst[:, sl], tmp)

                rope(kt)
                for h in range(NH):
                    rope(qt[:, h, :])

                # V^T -> V natural via PE transpose
                for st in range(NKT):
                    ptr = ps_o.tile([128, QB], F32, name="po")
                    nc.tensor.transpose(
                        ptr[:, 0:128], vt[:, st * 128 : (st + 1) * 128], ident
                    )
                    nc.vector.tensor_copy(vn[:, st, :], ptr[:, 0:128])

            if "B" not in phases:
                return
            # ---- Phase B: attention per (head, q-block), transposed layout ----
            with (
                tc.tile_pool(name="ptp", bufs=3) as ptp,
                tc.tile_pool(name="smallp", bufs=2) as smallp,
            ):
                for h in range(NH):
                    for jb in range(NQB):
                        nkt = 4 * (jb + 1)
                        psum_o = ps_o.tile([128, QB], F32, name="po")
                        acc = smallp.tile([128, QB], F32R, name="acc")
                        qslice = qt[:, h, jb * QB : (jb + 1) * QB]
                        for kti in range(nkt):
                            psum_s = ps_s.tile([128, QB], F32, name="pss")
                            nc.tensor.matmul(
                                psum_s,
                                kt[:, kti * 128 : (kti + 1) * 128],
                                qslice,
                                start=True,
                                stop=True,
                            )
                            rr = kti - 4 * jb
                            if rr >= 0:
                                nc.vector.tensor_add(
                                    psum_s, psum_s, mask_t[:, rr, :]
                                )
                            pt = ptp.tile([128, QB], F32R, name="pt")
                            nc.scalar.activation(
                                pt, psum_s, mybir.ActivationFunctionType.Exp
                            )
                            nc.tensor.matmul(
                                psum_o,
                                vn[:, kti, :],
                                pt,
                                start=(kti == 0),
                                stop=(kti == nkt - 1),
                            )
                            if kti == 0:
                                nc.vector.tensor_copy(acc, pt)
                            else:
                                nc.vector.tensor_add(acc, acc, pt)
                        psum_r = ps_sum.tile([1, QB], F32, name="psr")
                        nc.tensor.matmul(psum_r, ones, acc, start=True, stop=True)
                        rec = smallp.tile([1, QB], F32, name="rec")
                        nc.vector.reciprocal(rec, psum_r)
                        bcast = smallp.tile([128, QB], F32, name="bcast")
                        nc.gpsimd.partition_broadcast(bcast, rec)
                        # fused normalize + PSUM->SBUF, overwriting the spent
                        # q columns of head h
                        nc.vector.tensor_mul(qslice, psum_o, bcast)

            if "C" not in phases:
                return
            # ---- Phase C: o_proj + ReduceScatter per 512-wide D chunk ----
            with (
                tc.tile_pool(name="wop", bufs=9) as wop,
                tc.tile_pool(name="outp", bufs=3) as outp,
            ):
                for dc in range(NDC):
                    o_chunk = dram.tile([S, 512], F32, name=f"oc{dc}")
                    rs_chunk = dram.tile([512, 512], F32, name=f"rc{dc}")
                    wo_tiles = []
                    for ct in range(NH):
                        wt = wop.tile([128, 512], F32R, name="wo")
                        nc.sync.dma_start(out=wt, in_=wo_d[dc, ct])
                        wo_tiles.append(wt)
                    for q in range(NKT):
                        psum = ps_proj.tile([128, 512], F32, name="pp")
                        for ct in range(NH):
                            nc.tensor.matmul(
                                psum,
                                qt[:, ct, q * 128 : (q + 1) * 128],
                                wo_tiles[ct],
                                start=(ct == 0),
                                stop=(ct == NH - 1),
                            )
                        ob = outp.tile([128, 512], F32, name="ob")
                        nc.vector.tensor_copy(ob, psum)
                        nc.sync.dma_start(
                            out=o_chunk[q * 128 : (q + 1) * 128, :], in_=ob
                        )
                        if no_cc and q >= NKT - 4:
                            nc.sync.dma_start(
                                out=rs_out[dc, (q - (NKT - 4)) * 128 :
                                           (q - (NKT - 4)) * 128 + 128, :],
                                in_=ob,
                            )
                    if not no_cc:
                        nc.gpsimd.collective_compute(
                            "ReduceScatter",
                            mybir.AluOpType.add,
                            replica_groups=[[0, 1, 2, 3], [4, 5, 6, 7]],
                            ins=[o_chunk[:]],
                            outs=[rs_chunk[:]],
                        )
                        nc.sync.dma_start(out=rs_out[dc], in_=rs_chunk[:])


def _host_inputs(hidden_states, Wq, Wk, Wv, Wo):
    hidden = np.asarray(hidden_states, dtype=np.float32)
    Wq = np.asarray(Wq, dtype=np.float32) * np.float32(SCALE)
    Wk = np.asarray(Wk, dtype=np.float32)
    Wv = np.asarray(Wv, dtype=np.float32)
    Wo = np.asarray(Wo, dtype=np.float32)

    inv_freq = 1.0 / ROPE_THETA ** (np.arange(0, HD, 2, dtype=np.float32) / HD)
    t = np.arange(S, dtype=np.float32)
    freqs = np.outer(t, inv_freq)  # [S, 64]
    cos_t = np.cos(freqs.T)  # [64, S]
    sin_t = np.sin(freqs.T)
    cos_full = np.concatenate([cos_t, cos_t], axis=0).astype(np.float32)  # [128, S]
    sin_full = np.concatenate([-sin_t, sin_t], axis=0).astype(np.float32)

    krel = np.arange(512)[:, None]
    qrel = np.arange(QB)[None, :]
    mask = np.where(krel <= qrel, 0.0, -30000.0).astype(np.float32)
    mask = np.ascontiguousarray(mask.reshape(4, 128, QB))

    in_maps = []
    for core in range(8):
        b, r = core // 4, core % 4
        xt = np.ascontiguousarray(
            hidden[b].reshape(NSC, SC, DKT, 128).transpose(0, 2, 3, 1)
        )
        wq = np.ascontiguousarray(
            Wq[:, r * NH * HD : (r + 1) * NH * HD]
            .reshape(DKT, 128, NH, 128)
            .transpose(2, 1, 0, 3)
        )
        wk = np.ascontiguousarray(
            Wk[:, r * HD : (r + 1) * HD].reshape(DKT, 128, 128).transpose(1, 0, 2)
        )
        wv = np.ascontiguousarray(
            Wv[:, r * HD : (r + 1) * HD].reshape(DKT, 128, 128).transpose(1, 0, 2)
        )
        wo = np.ascontiguousarray(
            Wo[r * NH * HD : (r + 1) * NH * HD, :]
            .reshape(NH, 128, NDC, 512)
            .transpose(2, 0, 1, 3)
        )
        in_maps.append(
            {
                "xt": xt,
                "wq": wq,
                "wk": wk,
                "wv": wv,
                "wo": wo,
                "cos_t": cos_full,
                "sin_t": sin_full,
                "mask": mask,
            }
        )
    return in_maps


def kernel(hidden_states, Wq, Wk, Wv, Wo, trace=False):
    nc = _build_nc()
    in_maps = _host_inputs(hidden_states, Wq, Wk, Wv, Wo)
    res = run_bass_kernel_spmd(nc, in_maps, list(range(8)), trace=trace)
    out = np.empty((B, S, D), dtype=np.float32)
    for core in range(8):
        b, r = core // 4, core % 4
        rs = res.results[core]["rs_out"]  # [7, 512, 512]
        for dc in range(NDC):
            out[b, r * 512 : (r + 1) * 512, dc * 512 : (dc + 1) * 512] = rs[dc]
    if trace:
        kernel.last_exec_time_ns = res.exec_time_ns
    return out
